# revision 1
# baseline (speedup 1.0000x reference)
"""Causal attention kernel for 8 Trainium2 NeuronCores.

Problem: x[4, 4096, 512] @ {Wq,Wk,Wv}[512, 128] -> causal attention -> [4, 4096, 128].

Sharding: 2 cores per batch, interleaved over KEY chunks. Core c = 2b+p
(batch b, parity p) owns key chunks {2i+p : i=0..15} (chunks of 128 keys),
and computes, for every query block of its batch, the partial softmax
numerator and denominator over its keys. The host sums the two partials and
divides. Causality makes query block qg (512 rows) attend key chunks
0..4qg+3, of which each parity owns exactly 2qg+2 -> both cores run the
identical program (exact load balance); only the last two local chunks of
each block need a (parity-dependent, input-supplied) additive mask.

On-device layout: scores are computed transposed, S^T[key, q]:
  - K^T[d, key], Q^T[d, q] come from host-pre-transposed x (no on-device
    transposes anywhere); 1/sqrt(d) is folded into Wq on the host
  - S^T chunk = matmul(lhsT=K^T[:, chunk], rhs=Q^T[:, qblock])  [N=512]
  - E = exp(S^T + mask) via ScalarE, straight out of PSUM
  - numerator^T[e, q] accumulates in PSUM: matmul(lhsT=V[chunk], rhs=E)
  - denominator[1, q] accumulates in PSUM: matmul(lhsT=ones, rhs=E)

MODE selects matmul operand precision (PSUM accumulation is always fp32):
  "bf16" - operands bf16 (x/W cast on host, halving input DMA); fastest
  "f32r" - single-pass fp32 matmul, ~12-bit mantissa operands
  "f32"  - exact fp32 (2-pass LOW_HIGH matmuls), slowest
"""

import math

import numpy as np

B, S, DIN, DOUT = 4, 4096, 512, 128
NCORES = 8
TQ = 512            # query block size
NQB = S // TQ       # 8 query blocks per batch
KC = 128            # key chunk size
NKLOC = S // KC // 2  # 16 key chunks owned per core
NEG = -1.0e9

MODE = "bf16"

_cache = {}


def _np_in_dtype(mode):
    if mode == "bf16":
        import ml_dtypes

        return ml_dtypes.bfloat16
    return np.float32


def _build_nc(mode=None):
    import concourse.bacc as bacc
    import concourse.mybir as mybir
    import concourse.tile as tile

    mode = MODE if mode is None else mode
    f32 = mybir.dt.float32
    mdt = {
        "f32": f32,
        "f32r": mybir.dt.float32r,
        "bf16": mybir.dt.bfloat16,
    }[mode]

    nc = bacc.Bacc(None, target_bir_lowering=False, debug=False)

    NDC = DIN // 128    # 4 contraction chunks for the projections
    SK = NKLOC * KC     # 2048 owned keys
    WMC = 3 * NDC * DOUT + 2 * TQ  # packed w+masks columns

    # All inputs are host-packed partition-major so every DMA is contiguous:
    # xb[p, c, s] = x[b].T[c*128+p, s], wm[p, :] = [wq|wk|wv chunks, masks]
    xb_d = nc.declare_dram_parameter("xb", [128, NDC, S], mdt, isOutput=False)
    xk_d = nc.declare_dram_parameter("xk", [128, NDC, SK], mdt, isOutput=False)
    wm_d = nc.declare_dram_parameter("wm", [128, WMC], mdt, isOutput=False)
    numT = nc.declare_dram_parameter("numT", [DOUT, S], f32, isOutput=True)
    den = nc.declare_dram_parameter("den", [NQB, TQ], f32, isOutput=True)


    with tile.TileContext(nc) as tc:
        with (
            tc.tile_pool(name="persist", bufs=1) as persist,
            tc.tile_pool(name="pss", bufs=5, space="PSUM") as pss,
            tc.tile_pool(name="pso", bufs=2, space="PSUM") as pso,
            tc.tile_pool(name="psd", bufs=1, space="PSUM") as psd,
            tc.tile_pool(name="etile", bufs=10) as etile,
            tc.tile_pool(name="otile", bufs=2) as otile,
            tc.tile_pool(name="dtile", bufs=2) as dtile,
        ):
            # ---- resident SBUF tensors ----
            xb_t = persist.tile([128, NDC, S], mdt, tag="xb")
            xk_t = persist.tile([128, NDC, SK], mdt, tag="xk")
            wm_t = persist.tile([128, WMC], mdt, tag="wm")
            qT = persist.tile([128, S], mdt, tag="qT")
            kT = persist.tile([128, SK], mdt, tag="kT")
            v_t = persist.tile([128, NKLOC, DOUT], mdt, tag="v")

            def w_ap(wi, c):
                return wm_t[:, (wi * NDC + c) * DOUT:(wi * NDC + c + 1) * DOUT]

            def mask_ap(m):
                return wm_t[:, 3 * NDC * DOUT + m * TQ:3 * NDC * DOUT + (m + 1) * TQ]

            # Input DMA: issue order matters (each HWDGE ring is FIFO and a
            # dma_start occupies the ring ~0.6us regardless of size), so use
            # few, large DMAs, most-urgent first. sync ring: K/V-path inputs;
            # scalar ring: Q-path inputs. rearrange folds the DIN chunking
            # into a single access pattern.
            ones_f = persist.tile([128, 1], f32, tag="ones_f")
            nc.vector.memset(ones_f, 1.0)
            ones = persist.tile([128, 1], mdt, tag="ones")
            nc.vector.tensor_copy(ones[:], ones_f[:])

            nc.sync.dma_start(out=wm_t[:], in_=wm_d[:])
            # xk: small lead piece so K-proj slice 0 starts early, then halves
            for sl in (slice(0, 512), slice(512, SK // 2), slice(SK // 2, SK)):
                nc.sync.dma_start(out=xk_t[:, :, sl], in_=xk_d[:, :, sl])
            # xb on the scalar ring, progressive for Q-proj
            for sl in (
                slice(0, 512),
                slice(512, 1024),
                slice(1024, 2048),
                slice(2048, 3072),
                slice(3072, S),
            ):
                nc.scalar.dma_start(out=xb_t[:, :, sl], in_=xb_d[:, :, sl])

            # ---- projections (K/V first: attention consumes them earliest) ----
            for s512 in range(SK // 512):  # K^T over owned keys
                ps = pss.tile([128, 512], f32, tag="ps_s", name=f"psk{s512}")
                for c in range(NDC):
                    nc.tensor.matmul(
                        ps[:],
                        w_ap(1, c),
                        xk_t[:, c, s512 * 512:(s512 + 1) * 512],
                        start=(c == 0),
                        stop=(c == NDC - 1),
                    )
                nc.vector.tensor_copy(kT[:, s512 * 512:(s512 + 1) * 512], ps[:])
            for t in range(NKLOC):  # V[key, e] natural layout, owned keys
                ps = pss.tile([128, 512], f32, tag="ps_s", name=f"psv{t}")
                for c in range(NDC):
                    nc.tensor.matmul(
                        ps[:, :DOUT],
                        xk_t[:, c, t * KC:(t + 1) * KC],
                        w_ap(2, c),
                        start=(c == 0),
                        stop=(c == NDC - 1),
                    )
                nc.vector.tensor_copy(v_t[:, t, :], ps[:, :DOUT])
            for s512 in range(S // 512):  # Q^T over all queries, in block order
                ps = pss.tile([128, 512], f32, tag="ps_s", name=f"psq{s512}")
                for c in range(NDC):
                    nc.tensor.matmul(
                        ps[:],
                        w_ap(0, c),
                        xb_t[:, c, s512 * 512:(s512 + 1) * 512],
                        start=(c == 0),
                        stop=(c == NDC - 1),
                    )
                nc.vector.tensor_copy(qT[:, s512 * 512:(s512 + 1) * 512], ps[:])

            # ---- attention ----
            for qg in range(NQB):
                n_loc = 2 * qg + 2
                po = pso.tile([128, TQ], f32, tag="po", name=f"po{qg}")
                pd = psd.tile([1, TQ], f32, tag="pd", name=f"pd{qg}")
                for i in range(n_loc):
                    ps = pss.tile([128, TQ], f32, tag="ps_s", name=f"pss{qg}_{i}")
                    masked = i >= n_loc - 2
                    if masked:
                        # pre-bias PSUM with the additive causal mask (off the
                        # critical path), then accumulate scores onto it
                        nc.vector.tensor_copy(ps[:], mask_ap(i - (n_loc - 2)))
                    nc.tensor.matmul(
                        ps[:],
                        kT[:, i * KC:(i + 1) * KC],
                        qT[:, qg * TQ:(qg + 1) * TQ],
                        start=not masked,
                        stop=True,
                    )
                    e = etile.tile([128, TQ], mdt, tag="e", name=f"e{qg}_{i}")
                    nc.scalar.activation(
                        e[:], ps[:], mybir.ActivationFunctionType.Exp
                    )
                    nc.tensor.matmul(
                        po[:],
                        v_t[:, i, :],
                        e[:],
                        start=(i == 0),
                        stop=(i == n_loc - 1),
                    )
                    nc.tensor.matmul(
                        pd[:],
                        ones[:],
                        e[:],
                        start=(i == 0),
                        stop=(i == n_loc - 1),
                    )
                o = otile.tile([128, TQ], f32, tag="o", name=f"o{qg}")
                nc.vector.tensor_copy(o[:], po[:])
                nc.scalar.dma_start(out=numT[:, qg * TQ:(qg + 1) * TQ], in_=o[:])
                d = dtile.tile([1, TQ], f32, tag="d", name=f"d{qg}")
                nc.vector.tensor_copy(d[:], pd[:])
                nc.sync.dma_start(out=den[qg:qg + 1, :], in_=d[:])

    nc.finalize()
    return nc


def _owned_keys(par):
    return np.concatenate(
        [np.arange((2 * i + par) * KC, (2 * i + par) * KC + KC) for i in range(NKLOC)]
    )


def _build_masks(par):
    # last two local chunks of each query block: relative chunk r0 = par,
    # r1 = 2 + par; element [k, q] allowed iff 128*r + k <= q
    r = np.array([par, 2 + par])[:, None, None]
    k = np.arange(KC)[None, :, None]
    q = np.arange(TQ)[None, None, :]
    allowed = (KC * r + k) <= q
    return np.where(allowed, np.float32(0.0), np.float32(NEG)).astype(np.float32)


def _get_nc():
    if "nc" not in _cache:
        _cache["nc"] = _build_nc()
    return _cache["nc"]


def _pack_pm(a):
    # [DIN, cols] -> partition-major [128, DIN//128, cols]
    return np.ascontiguousarray(a.reshape(DIN // 128, 128, a.shape[1]).transpose(1, 0, 2))


def _prepare_in_maps(x, Wq, Wk, Wv, mode=None):
    mode = MODE if mode is None else mode
    idt = _np_in_dtype(mode)
    ws = [(Wq / math.sqrt(DOUT)).astype(idt), Wk.astype(idt), Wv.astype(idt)]
    w_pack = np.concatenate(
        [_pack_pm(w).reshape(128, -1) for w in ws], axis=1
    )  # [128, 1536]
    in_maps = []
    for c in range(NCORES):
        b, par = c // 2, c % 2
        xbt = x[b].T.astype(idt)
        m = _build_masks(par).astype(idt)  # [2, 128, 512]
        wm = np.concatenate(
            [w_pack, np.ascontiguousarray(m.transpose(1, 0, 2)).reshape(128, -1)],
            axis=1,
        )
        in_maps.append({
            "xb": _pack_pm(xbt),
            "xk": _pack_pm(np.ascontiguousarray(xbt[:, _owned_keys(par)])),
            "wm": np.ascontiguousarray(wm),
        })
    return in_maps


def _gather(results):
    out = np.empty((B, S, DOUT), dtype=np.float32)
    for b in range(B):
        r0, r1 = results[2 * b], results[2 * b + 1]
        num = r0["numT"].astype(np.float64).T + r1["numT"].astype(np.float64).T
        d = r0["den"].astype(np.float64).reshape(-1) + r1["den"].astype(
            np.float64
        ).reshape(-1)
        out[b] = (num / d[:, None]).astype(np.float32)
    return out


def kernel(**inputs):
    from concourse.bass_utils import run_bass_kernel_spmd

    x = np.asarray(inputs["x"], dtype=np.float32)
    Wq = np.asarray(inputs["Wq"], dtype=np.float32)
    Wk = np.asarray(inputs["Wk"], dtype=np.float32)
    Wv = np.asarray(inputs["Wv"], dtype=np.float32)

    nc = _get_nc()
    in_maps = _prepare_in_maps(x, Wq, Wk, Wv)
    res = run_bass_kernel_spmd(nc, in_maps, list(range(NCORES)))
    return _gather(res.results)



# revision 12
# speedup vs baseline: 1.1698x; 1.1698x over previous
"""Causal attention kernel for 8 Trainium2 NeuronCores.

Problem: x[4, 4096, 512] @ {Wq,Wk,Wv}[512, 128] -> causal attention -> [4, 4096, 128].

Sharding: 2 cores per batch, interleaved over KEY chunks. Core c = 2b+p
(batch b, parity p) owns key chunks {2i+p : i=0..15} (chunks of 128 keys),
and computes, for every query block of its batch, the partial softmax
numerator and denominator over its keys. The host sums the two partials and
divides. Causality makes query block qg (512 rows) attend key chunks
0..4qg+3, of which each parity owns exactly 2qg+2 -> both cores run the
identical program (exact load balance); only the last two local chunks of
each block need a (parity-dependent, input-supplied) additive mask.

On-device layout: scores are computed transposed, S^T[key, q]:
  - K^T[d, key], Q^T[d, q] from host-pre-transposed x; no on-device transposes
  - S^T chunk = matmul(lhsT=K^T[:, chunk], rhs=Q^T[:, qblock])  [N=512]
  - E = exp(scale * S^T_raw) via one ScalarE activation per chunk-PAIR
    ([128, 1024] from 2 PSUM banks), output fp8
  - numerator^T[e, q] accumulates in PSUM via fp8 DoubleRow matmuls
    (each contracts a 256-key chunk pair in one 512-col stream)
  - denominator[1, q] likewise via DoubleRow ones-matmuls

Precision plan (rel-err budget 2e-2):
  - x and 16*W shipped fp8e4 (x16 scaling keeps W out of e4m3 subnormals);
    Q/K projections are fp8 DoubleRow matmuls contracting 256 din/instr.
    The exp activation's free scale parameter applies 1/(256*sqrt(128)).
  - scores matmuls bf16 (kT/qT are bf16 copies of the PSUM projections).
  - block 0 (queries 0..511) uses a bf16 V path + bf16 E: its early rows
    have near-delta attention where fp8 V error would hit the output scale
    directly. Elsewhere fp8 V/E noise averages out over >=513 keys.
  - numerator output bf16, denominator f32; host divides in f64.
"""

import math

import numpy as np

B, S, DIN, DOUT = 4, 4096, 512, 128
NCORES = 8
TQ = 512             # query block size
NQB = S // TQ        # 8 query blocks per batch
KC = 128             # key chunk size
NKLOC = S // KC // 2  # 16 key chunks owned per core
SK = NKLOC * KC      # 2048 owned keys
NDC = DIN // 128     # 4 contraction chunks
NEG = -1.0e6         # additive mask in raw-score units (pre exp-scale)
WS = 16.0            # host-side weight prescale (fp8 subnormal avoidance)
SCALE = 1.0 / (WS * WS * math.sqrt(DOUT))  # exp(scale * raw_score)
WM16C = 2 * TQ + NDC * DOUT  # masks (2*512) + bf16 Wv (4*128)

_cache = {}


def _build_nc():
    import concourse.bacc as bacc
    import concourse.mybir as mybir
    import concourse.tile as tile

    f32 = mybir.dt.float32
    bf16 = mybir.dt.bfloat16
    f8 = mybir.dt.float8e4
    DR = mybir.MatmulPerfMode.DoubleRow
    Exp = mybir.ActivationFunctionType.Exp

    nc = bacc.Bacc(None, target_bir_lowering=False, debug=False)

    xb8_d = nc.declare_dram_parameter("xb8", [128, NDC, S], f8, isOutput=False)
    xk8_d = nc.declare_dram_parameter("xk8", [128, NDC, SK], f8, isOutput=False)
    xe_d = nc.declare_dram_parameter("xe", [128, NDC, 2 * KC], bf16, isOutput=False)
    wm8_d = nc.declare_dram_parameter("wm8", [128, 3, NDC, DOUT], f8, isOutput=False)
    wm16_d = nc.declare_dram_parameter("wm16", [128, WM16C], bf16, isOutput=False)
    numT_d = nc.declare_dram_parameter("numT", [DOUT, S], bf16, isOutput=True)
    den_d = nc.declare_dram_parameter("den", [NQB, TQ], f32, isOutput=True)

    with tile.TileContext(nc) as tc:
        with (
            tc.tile_pool(name="persist", bufs=1) as persist,
            tc.tile_pool(name="pss", bufs=3, space="PSUM") as pss,
            tc.tile_pool(name="pso", bufs=1, space="PSUM") as pso,
            tc.tile_pool(name="psd", bufs=1, space="PSUM") as psd,
            tc.tile_pool(name="et", bufs=4) as et,
            tc.tile_pool(name="dt", bufs=2) as dt,
        ):
            xb8_t = persist.tile([128, NDC, S], f8, tag="xb8")
            xk8_t = persist.tile([128, NDC, SK], f8, tag="xk8")
            xe_t = persist.tile([128, NDC, 2 * KC], bf16, tag="xe")
            wm8_t = persist.tile([128, 3, NDC, DOUT], f8, tag="wm8")
            wm16_t = persist.tile([128, WM16C], bf16, tag="wm16")
            qT = persist.tile([128, S], bf16, tag="qT")
            kT = persist.tile([128, SK], bf16, tag="kT")
            v8 = persist.tile([128, NKLOC, DOUT], f8, tag="v8")
            v16 = persist.tile([128, 2, DOUT], bf16, tag="v16")
            onum = persist.tile([128, S], bf16, tag="onum")
            ones8 = persist.tile([128, 2, 16], f8, tag="ones8")
            ones16 = persist.tile([128, 1], bf16, tag="ones16")
            warm_i = persist.tile([1, 16], f32, tag="warm_i")
            warm_o = persist.tile([1, 16], bf16, tag="warm_o")

            def w8(wi, p):  # fp8 weight chunk pair [128, 2, DOUT]
                return wm8_t[:, wi, 2 * p:2 * p + 2, :]

            def mask_ap():  # additive causal masks for the diagonal pair
                return wm16_t[:, 0:2 * TQ]

            def wv16(c):  # bf16 Wv chunk for the block-0 V path
                return wm16_t[:, 2 * TQ + c * DOUT:2 * TQ + (c + 1) * DOUT]

            # Exp-table warmup on ScalarE: the ~2.7us ACT_TABLE_LOAD happens
            # here, overlapped with input DMA, instead of at the first score.
            nc.vector.memset(warm_i, 0.0)
            nc.scalar.activation(warm_o[:], warm_i[:], Exp, scale=SCALE)
            nc.vector.memset(ones8, 1.0)
            nc.vector.memset(ones16, 1.0)

            # Input DMA on the sync ring, most-urgent first; stage qg needs
            # xk8 cols < 256(qg+1) and xb8 cols < 512(qg+1).
            nc.sync.dma_start(out=wm8_t[:], in_=wm8_d[:])
            nc.sync.dma_start(out=xk8_t[:, :, 0:2 * KC], in_=xk8_d[:, :, 0:2 * KC])
            nc.sync.dma_start(out=xb8_t[:, :, 0:TQ], in_=xb8_d[:, :, 0:TQ])
            nc.sync.dma_start(out=wm16_t[:], in_=wm16_d[:])
            nc.sync.dma_start(out=xe_t[:], in_=xe_d[:])
            nc.sync.dma_start(out=xk8_t[:, :, 2 * KC:6 * KC], in_=xk8_d[:, :, 2 * KC:6 * KC])
            nc.sync.dma_start(out=xb8_t[:, :, TQ:3 * TQ], in_=xb8_d[:, :, TQ:3 * TQ])
            nc.sync.dma_start(out=xk8_t[:, :, 6 * KC:SK], in_=xk8_d[:, :, 6 * KC:SK])
            nc.sync.dma_start(out=xb8_t[:, :, 3 * TQ:5 * TQ], in_=xb8_d[:, :, 3 * TQ:5 * TQ])
            nc.sync.dma_start(out=xb8_t[:, :, 5 * TQ:S], in_=xb8_d[:, :, 5 * TQ:S])

            ones8_ap = ones8[:, :, 0:1]  # [128, 2, 1], slot step 16B

            for qg in range(NQB):
                # ---- projection stage qg: K/V local chunks {2qg, 2qg+1} + Q block qg
                # one 2-bank psum tile: bank0 = K(2x128)|V(2x128), bank1 = Q(512).
                # Each column region is its own start..stop group: a start
                # clears the whole bank's has_written bits, but closed groups'
                # data is never accumulated again so that is harmless. The v8
                # cast depends on the bank's LAST writer and the kT cast
                # follows it on the FIFO vector queue, so no DVE read overlaps
                # a PE write to the same bank.
                ps = pss.tile([128, 2, TQ], f32, tag="ps", name=f"psj{qg}")
                for p in range(2):
                    nc.tensor.matmul(
                        ps[:, 1, :],
                        w8(0, p),
                        xb8_t[:, 2 * p:2 * p + 2, qg * TQ:(qg + 1) * TQ],
                        start=(p == 0), stop=(p == 1), perf_mode=DR,
                    )
                for j in range(2):
                    kcols = slice((2 * qg + j) * KC, (2 * qg + j + 1) * KC)
                    for p in range(2):
                        nc.tensor.matmul(
                            ps[:, 0, j * KC:(j + 1) * KC],
                            w8(1, p),
                            xk8_t[:, 2 * p:2 * p + 2, kcols],
                            start=(p == 0), stop=(p == 1), perf_mode=DR,
                        )
                for j in range(2):
                    kcols = slice((2 * qg + j) * KC, (2 * qg + j + 1) * KC)
                    for c in range(NDC):
                        nc.tensor.matmul(
                            ps[:, 0, 2 * KC + j * KC:2 * KC + (j + 1) * KC],
                            xk8_t[:, c, kcols],
                            wm8_t[:, 2, c, :],
                            start=(c == 0), stop=(c == NDC - 1),
                        )
                nc.vector.tensor_copy(v8[:, 2 * qg:2 * qg + 2, :], ps[:, 0, 2 * KC:4 * KC])
                nc.vector.tensor_copy(kT[:, 2 * qg * KC:(2 * qg + 2) * KC], ps[:, 0, 0:2 * KC])
                nc.vector.tensor_copy(qT[:, qg * TQ:(qg + 1) * TQ], ps[:, 1, :])

                if qg == 0:
                    # bf16 V for the two early owned chunks (keys < 512)
                    pv = pss.tile([128, 2, TQ], f32, tag="ps", name="psv16")
                    for j in range(2):
                        for c in range(NDC):
                            nc.tensor.matmul(
                                pv[:, 0, j * KC:(j + 1) * KC],
                                xe_t[:, c, j * KC:(j + 1) * KC],
                                wv16(c),
                                start=(c == 0), stop=(c == NDC - 1),
                            )
                    nc.vector.tensor_copy(v16[:, :, :], pv[:, 0, 0:2 * KC])

                # ---- attention block qg over qg+1 owned chunk pairs
                po = pso.tile([128, TQ], f32, tag="po", name=f"po{qg}")
                pd = psd.tile([1, TQ], f32, tag="pd", name=f"pd{qg}")
                qs = qT[:, qg * TQ:(qg + 1) * TQ]
                n_pair = qg + 1
                for p in range(n_pair):
                    masked = (p == n_pair - 1)
                    sp = pss.tile([128, 2, TQ], f32, tag="ps", name=f"sp{qg}_{p}")
                    if masked:
                        # pre-bias PSUM with the additive causal mask, then
                        # accumulate raw scores onto it (start=False)
                        nc.vector.tensor_copy(sp[:], mask_ap())
                    for s in range(2):
                        # diagonal pair slot 1 (relative chunk 2+par) is fully
                        # masked for queries < 256 in both parities: skip those
                        # columns, the mask pre-bias already holds NEG there
                        c0 = TQ // 2 if (masked and s == 1) else 0
                        nc.tensor.matmul(
                            sp[:, s, c0:],
                            kT[:, (2 * p + s) * KC:(2 * p + s + 1) * KC],
                            qs[:, c0:],
                            start=not masked, stop=True,
                            skip_group_check=masked,
                        )
                    if qg == 0:
                        e = et.tile([128, 2, TQ], bf16, tag="e16", bufs=1, name="e0")
                        nc.scalar.activation(e[:], sp[:], Exp, scale=SCALE)
                        for s in range(2):
                            nc.tensor.matmul(po[:], v16[:, s, :], e[:, s, :],
                                             start=(s == 0), stop=(s == 1))
                        for s in range(2):
                            nc.tensor.matmul(pd[:], ones16[:], e[:, s, :],
                                             start=(s == 0), stop=(s == 1))
                    else:
                        e = et.tile([128, 2, TQ], f8, tag="e8", name=f"e{qg}_{p}")
                        nc.scalar.activation(e[:], sp[:], Exp, scale=SCALE)
                        nc.tensor.matmul(po[:], v8[:, 2 * p:2 * p + 2, :], e[:],
                                         start=(p == 0), stop=(p == n_pair - 1),
                                         perf_mode=DR)
                        nc.tensor.matmul(pd[:], ones8_ap, e[:],
                                         start=(p == 0), stop=(p == n_pair - 1),
                                         perf_mode=DR)
                nc.vector.tensor_copy(onum[:, qg * TQ:(qg + 1) * TQ], po[:])
                d = dt.tile([1, TQ], f32, tag="d", name=f"d{qg}")
                nc.vector.tensor_copy(d[:], pd[:])
                nc.sync.dma_start(out=den_d[qg:qg + 1, :], in_=d[:])
                if qg % 2 == 1:
                    nc.sync.dma_start(
                        out=numT_d[:, (qg - 1) * TQ:(qg + 1) * TQ],
                        in_=onum[:, (qg - 1) * TQ:(qg + 1) * TQ],
                    )

    nc.finalize()
    return nc


def _owned_keys(par):
    return np.concatenate(
        [np.arange((2 * i + par) * KC, (2 * i + par) * KC + KC) for i in range(NKLOC)]
    )


def _build_masks(par):
    # diagonal pair of each query block: slot s in {0,1} is relative chunk
    # r = 2s + par; element [k, q] allowed iff 128*r + k <= q
    r = np.array([par, 2 + par])[:, None, None]
    k = np.arange(KC)[None, :, None]
    q = np.arange(TQ)[None, None, :]
    allowed = (KC * r + k) <= q
    return np.where(allowed, np.float32(0.0), np.float32(NEG)).astype(np.float32)


def _get_nc():
    if "nc" not in _cache:
        _cache["nc"] = _build_nc()
    return _cache["nc"]


def _pack_pm(a):
    # [DIN, cols] -> partition-major [128, DIN//128, cols]
    return np.ascontiguousarray(a.reshape(DIN // 128, 128, a.shape[1]).transpose(1, 0, 2))


def _prepare_in_maps(x, Wq, Wk, Wv):
    import ml_dtypes

    f8 = ml_dtypes.float8_e4m3
    bf16 = ml_dtypes.bfloat16

    # fp8 weights: [128, 3, NDC, DOUT]; Wq/Wk prescaled by WS, Wv unscaled
    w8_pack = np.stack(
        [_pack_pm(np.clip(w * s, -240, 240).astype(np.float32))
         for w, s in ((Wq, WS), (Wk, WS), (Wv, 1.0))],
        axis=1,
    ).astype(f8)  # [128, 3, NDC, DOUT]

    wv16_pack = _pack_pm(Wv).reshape(128, NDC * DOUT).astype(bf16)

    in_maps = []
    for c in range(NCORES):
        b, par = c // 2, c % 2
        xbt = np.ascontiguousarray(x[b].T)
        owned = _owned_keys(par)
        m = _build_masks(par)  # [2, 128, 512]
        m16 = np.ascontiguousarray(m.transpose(1, 0, 2)).reshape(128, 2 * TQ)
        wm16 = np.concatenate([m16, np.zeros((128, NDC * DOUT), np.float32)], axis=1)
        wm16 = wm16.astype(bf16)
        wm16[:, 2 * TQ:] = wv16_pack
        in_maps.append({
            "xb8": _pack_pm(xbt).astype(f8),
            "xk8": _pack_pm(np.ascontiguousarray(xbt[:, owned])).astype(f8),
            "xe": _pack_pm(np.ascontiguousarray(xbt[:, owned[:2 * KC]])).astype(bf16),
            "wm8": w8_pack,
            "wm16": wm16,
        })
    return in_maps


def _gather(results):
    out = np.empty((B, S, DOUT), dtype=np.float32)
    for b in range(B):
        r0, r1 = results[2 * b], results[2 * b + 1]
        num = r0["numT"].astype(np.float64).T + r1["numT"].astype(np.float64).T
        d = r0["den"].astype(np.float64).reshape(-1) + r1["den"].astype(
            np.float64
        ).reshape(-1)
        out[b] = (num / d[:, None]).astype(np.float32)
    return out


def kernel(**inputs):
    from concourse.bass_utils import run_bass_kernel_spmd

    x = np.asarray(inputs["x"], dtype=np.float32)
    Wq = np.asarray(inputs["Wq"], dtype=np.float32)
    Wk = np.asarray(inputs["Wk"], dtype=np.float32)
    Wv = np.asarray(inputs["Wv"], dtype=np.float32)

    nc = _get_nc()
    in_maps = _prepare_in_maps(x, Wq, Wk, Wv)
    res = run_bass_kernel_spmd(nc, in_maps, list(range(NCORES)))
    return _gather(res.results)


# revision 16
# speedup vs baseline: 1.4725x; 1.2588x over previous
"""Causal attention kernel for 8 Trainium2 NeuronCores.

Problem: x[4, 4096, 512] @ {Wq,Wk,Wv}[512, 128] -> causal attention -> [4, 4096, 128].

Sharding: 2 cores per batch, interleaved over KEY chunks. Core c = 2b+p
(batch b, parity p) owns key chunks {2i+p : i=0..15} (chunks of 128 keys),
and computes, for every query block of its batch, the partial softmax
numerator and denominator over its keys. The host sums the two partials and
divides. Causality makes query block qg (512 rows) attend key chunks
0..4qg+3, of which each parity owns exactly 2qg+2 -> both cores run the
identical program (exact load balance); only the last two local chunks of
each block need a (parity-dependent, input-supplied) additive mask.

On-device layout: scores are computed transposed, S^T[key, q]:
  - K^T[d, key], Q^T[d, q] from host-pre-transposed x; no on-device transposes
  - S^T chunk = matmul(lhsT=K^T[:, chunk], rhs=Q^T[:, qblock])  [N=512]
  - E = exp(scale * S^T_raw) via one ScalarE activation per chunk-PAIR
    ([128, 1024] from 2 PSUM banks), output fp8
  - numerator^T[e, q] accumulates in PSUM via fp8 DoubleRow matmuls
    (each contracts a 256-key chunk pair in one 512-col stream)
  - denominator[1, q] likewise via DoubleRow ones-matmuls

Precision plan (rel-err budget 2e-2):
  - x and 16*W shipped fp8e4 (x16 scaling keeps W out of e4m3 subnormals);
    Q/K projections are fp8 DoubleRow matmuls contracting 256 din/instr.
    The exp activation's free scale parameter applies 1/(256*sqrt(128)).
  - scores matmuls bf16 (kT/qT are bf16 copies of the PSUM projections).
  - block 0 (queries 0..511) uses a bf16 V path + bf16 E: its early rows
    have near-delta attention where fp8 V error would hit the output scale
    directly. Elsewhere fp8 V/E noise averages out over >=513 keys.
  - numerator output bf16, denominator f32; host divides in f64.
"""

import math

import numpy as np

B, S, DIN, DOUT = 4, 4096, 512, 128
NCORES = 8
TQ = 512             # query block size
NQB = S // TQ        # 8 query blocks per batch
KC = 128             # key chunk size
NKLOC = S // KC // 2  # 16 key chunks owned per core
SK = NKLOC * KC      # 2048 owned keys
NDC = DIN // 128     # 4 contraction chunks
NEG = -1.0e6         # additive mask in raw-score units (pre exp-scale)
WS = 16.0            # host-side weight prescale (fp8 subnormal avoidance)
SCALE = 1.0 / (WS * WS * math.sqrt(DOUT))  # exp(scale * raw_score)
WM16C = 2 * TQ + NDC * DOUT  # masks (2*512) + bf16 Wv (4*128)

_cache = {}


def _build_nc():
    import concourse.bacc as bacc
    import concourse.mybir as mybir
    import concourse.tile as tile

    f32 = mybir.dt.float32
    bf16 = mybir.dt.bfloat16
    f8 = mybir.dt.float8e4
    DR = mybir.MatmulPerfMode.DoubleRow
    Exp = mybir.ActivationFunctionType.Exp

    nc = bacc.Bacc(None, target_bir_lowering=False, debug=False)

    xb8_d = nc.declare_dram_parameter("xb8", [128, NDC, S], f8, isOutput=False)
    xk8_d = nc.declare_dram_parameter("xk8", [128, NDC, SK], f8, isOutput=False)
    xe_d = nc.declare_dram_parameter("xe", [128, NDC, 2 * KC], bf16, isOutput=False)
    wm8_d = nc.declare_dram_parameter("wm8", [128, 3, NDC, DOUT], f8, isOutput=False)
    wm16_d = nc.declare_dram_parameter("wm16", [128, WM16C], bf16, isOutput=False)
    numT_d = nc.declare_dram_parameter("numT", [DOUT, S], bf16, isOutput=True)
    den_d = nc.declare_dram_parameter("den", [NQB, TQ], f32, isOutput=True)

    with tile.TileContext(nc) as tc:
        with (
            tc.tile_pool(name="persist", bufs=1) as persist,
            tc.tile_pool(name="pss", bufs=3, space="PSUM") as pss,
            tc.tile_pool(name="pso", bufs=1, space="PSUM") as pso,
            tc.tile_pool(name="psd", bufs=1, space="PSUM") as psd,
            tc.tile_pool(name="et", bufs=4) as et,
            tc.tile_pool(name="dt", bufs=2) as dt,
        ):
            xb8_t = persist.tile([128, NDC, S], f8, tag="xb8")
            xk8_t = persist.tile([128, NDC, SK], f8, tag="xk8")
            xe_t = persist.tile([128, NDC, 2 * KC], bf16, tag="xe")
            wm8_t = persist.tile([128, 3, NDC, DOUT], f8, tag="wm8")
            wm16_t = persist.tile([128, WM16C], bf16, tag="wm16")
            qT = persist.tile([128, S], bf16, tag="qT")
            kT = persist.tile([128, SK], bf16, tag="kT")
            v8 = persist.tile([128, NKLOC, DOUT], f8, tag="v8")
            v16 = persist.tile([128, 2, DOUT], bf16, tag="v16")
            onum = persist.tile([128, S], bf16, tag="onum")
            ones8 = persist.tile([128, 2, 16], f8, tag="ones8")
            ones16 = persist.tile([128, 1], bf16, tag="ones16")
            warm_i = persist.tile([1, 16], f32, tag="warm_i")
            warm_o = persist.tile([1, 16], bf16, tag="warm_o")

            def w8(wi, p):  # fp8 weight chunk pair [128, 2, DOUT]
                return wm8_t[:, wi, 2 * p:2 * p + 2, :]

            def mask_ap():  # additive causal masks for the diagonal pair
                return wm16_t[:, 0:2 * TQ]

            def wv16(c):  # bf16 Wv chunk for the block-0 V path
                return wm16_t[:, 2 * TQ + c * DOUT:2 * TQ + (c + 1) * DOUT]

            # Exp-table warmup on ScalarE: the ~2.7us ACT_TABLE_LOAD happens
            # here, overlapped with input DMA, instead of at the first score.
            nc.vector.memset(warm_i, 0.0)
            nc.scalar.activation(warm_o[:], warm_i[:], Exp, scale=SCALE)
            nc.vector.memset(ones8, 1.0)
            nc.vector.memset(ones16, 1.0)

            # Input DMA on the sync ring, just-in-time per stage: stage sg
            # needs xk8 cols < 256(sg+1) and xb8 cols < 512(sg+1). All input
            # dma_starts are dependency-free, so they never block the ring.
            def dma_stage_inputs(sg):
                nc.sync.dma_start(
                    out=xk8_t[:, :, sg * 2 * KC:(sg + 1) * 2 * KC],
                    in_=xk8_d[:, :, sg * 2 * KC:(sg + 1) * 2 * KC],
                )
                nc.sync.dma_start(
                    out=xb8_t[:, :, sg * TQ:(sg + 1) * TQ],
                    in_=xb8_d[:, :, sg * TQ:(sg + 1) * TQ],
                )

            nc.sync.dma_start(out=wm8_t[:], in_=wm8_d[:])
            dma_stage_inputs(0)
            dma_stage_inputs(1)
            nc.sync.dma_start(out=xe_t[:], in_=xe_d[:])
            nc.sync.dma_start(out=wm16_t[:], in_=wm16_d[:])
            for sg in range(2, NQB):
                dma_stage_inputs(sg)

            ones8_ap = ones8[:, :, 0:1]  # [128, 2, 1], slot step 16B

            def emit_stage(sg):
                # ---- projection stage sg: K/V local chunks {2sg, 2sg+1} + Q block sg
                # one 2-bank psum tile: bank0 = V(2x128)|K(2x128), bank1 = Q(512).
                # Each column region is its own start..stop group: a start
                # clears the whole bank's has_written bits, but closed groups'
                # data is never accumulated again so that is harmless. PE write
                # order is V, K, Q; DVE cast order is kT (depends on the last
                # bank-0 writer), qT, v8 — so no DVE read overlaps a PE write
                # to the same bank, and the casts the next block needs first
                # are first in the FIFO.
                ps = pss.tile([128, 2, TQ], f32, tag="ps", name=f"psj{sg}")
                for j in range(2):
                    kcols = slice((2 * sg + j) * KC, (2 * sg + j + 1) * KC)
                    for c in range(NDC):
                        nc.tensor.matmul(
                            ps[:, 0, 2 * KC + j * KC:2 * KC + (j + 1) * KC],
                            xk8_t[:, c, kcols],
                            wm8_t[:, 2, c, :],
                            start=(c == 0), stop=(c == NDC - 1),
                        )
                for j in range(2):
                    kcols = slice((2 * sg + j) * KC, (2 * sg + j + 1) * KC)
                    for p in range(2):
                        nc.tensor.matmul(
                            ps[:, 0, j * KC:(j + 1) * KC],
                            w8(1, p),
                            xk8_t[:, 2 * p:2 * p + 2, kcols],
                            start=(p == 0), stop=(p == 1), perf_mode=DR,
                        )
                for p in range(2):
                    nc.tensor.matmul(
                        ps[:, 1, :],
                        w8(0, p),
                        xb8_t[:, 2 * p:2 * p + 2, sg * TQ:(sg + 1) * TQ],
                        start=(p == 0), stop=(p == 1), perf_mode=DR,
                    )
                nc.vector.tensor_copy(kT[:, 2 * sg * KC:(2 * sg + 2) * KC], ps[:, 0, 0:2 * KC])
                nc.vector.tensor_copy(qT[:, sg * TQ:(sg + 1) * TQ], ps[:, 1, :])
                nc.vector.tensor_copy(v8[:, 2 * sg:2 * sg + 2, :], ps[:, 0, 2 * KC:4 * KC])

            def emit_v16():
                # bf16 V for the two early owned chunks (keys < 512)
                pv = pss.tile([128, 2, TQ], f32, tag="ps", name="psv16")
                for j in range(2):
                    for c in range(NDC):
                        nc.tensor.matmul(
                            pv[:, 0, j * KC:(j + 1) * KC],
                            xe_t[:, c, j * KC:(j + 1) * KC],
                            wv16(c),
                            start=(c == 0), stop=(c == NDC - 1),
                        )
                nc.vector.tensor_copy(v16[:, :, :], pv[:, 0, 0:2 * KC])

            def emit_block(qg):
                # ---- attention block qg over qg+1 owned chunk pairs
                po = pso.tile([128, TQ], f32, tag="po", name=f"po{qg}")
                pd = psd.tile([1, TQ], f32, tag="pd", name=f"pd{qg}")
                qs = qT[:, qg * TQ:(qg + 1) * TQ]
                n_pair = qg + 1
                for p in range(n_pair):
                    masked = (p == n_pair - 1)
                    sp = pss.tile([128, 2, TQ], f32, tag="ps", name=f"sp{qg}_{p}")
                    if masked:
                        # diagonal pair slot 1 (relative chunk 2+par) is fully
                        # masked for queries < 256 in both parities: memset
                        # NEG there (off the critical path) and skip those
                        # columns in the matmul below
                        nc.vector.memset(sp[:, 1, 0:TQ // 2], NEG)
                    for s in range(2):
                        c0 = TQ // 2 if (masked and s == 1) else 0
                        nc.tensor.matmul(
                            sp[:, s, c0:],
                            kT[:, (2 * p + s) * KC:(2 * p + s + 1) * KC],
                            qs[:, c0:],
                            start=True, stop=True,
                        )
                    if masked:
                        # add the causal mask AFTER the matmuls (a plain DVE
                        # read-modify-write with ordinary tile dependencies;
                        # PSUM pre-bias + start=False accumulation is fragile:
                        # has_written bits are only set by PE writes, so the
                        # accumulate-vs-overwrite choice would depend on stale
                        # per-bank state from the slot's previous tile)
                        nc.vector.tensor_add(
                            sp[:, 0, 0:2 * KC], sp[:, 0, 0:2 * KC],
                            wm16_t[:, 0:2 * KC],
                        )
                        nc.vector.tensor_add(
                            sp[:, 1, TQ // 2:], sp[:, 1, TQ // 2:],
                            wm16_t[:, TQ + TQ // 2:2 * TQ],
                        )
                    if qg == 0:
                        e = et.tile([128, 2, TQ], bf16, tag="e16", bufs=1, name="e0")
                        nc.scalar.activation(e[:], sp[:], Exp, scale=SCALE)
                        for s in range(2):
                            nc.tensor.matmul(po[:], v16[:, s, :], e[:, s, :],
                                             start=(s == 0), stop=(s == 1))
                        for s in range(2):
                            nc.tensor.matmul(pd[:], ones16[:], e[:, s, :],
                                             start=(s == 0), stop=(s == 1))
                    else:
                        e = et.tile([128, 2, TQ], f8, tag="e8", name=f"e{qg}_{p}")
                        nc.scalar.activation(e[:], sp[:], Exp, scale=SCALE)
                        nc.tensor.matmul(po[:], v8[:, 2 * p:2 * p + 2, :], e[:],
                                         start=(p == 0), stop=(p == n_pair - 1),
                                         perf_mode=DR)
                        nc.tensor.matmul(pd[:], ones8_ap, e[:],
                                         start=(p == 0), stop=(p == n_pair - 1),
                                         perf_mode=DR)
                # output copy on ScalarE (it has idle slots between ACTs);
                # keeps the next block's kT/qT casts unblocked on the DVE FIFO
                nc.scalar.copy(onum[:, qg * TQ:(qg + 1) * TQ], po[:])
                d = dt.tile([1, TQ], f32, tag="d", name=f"d{qg}")
                nc.vector.tensor_copy(d[:], pd[:])
                nc.sync.dma_start(out=den_d[qg:qg + 1, :], in_=d[:])
                nc.sync.dma_start(
                    out=numT_d[:, qg * TQ:(qg + 1) * TQ],
                    in_=onum[:, qg * TQ:(qg + 1) * TQ],
                )

            # software pipeline: emit stage sg one block ahead of block sg,
            # so its PSUM->SBUF casts happen while the previous block computes
            emit_stage(0)
            emit_stage(1)
            emit_v16()
            for qg in range(NQB):
                emit_block(qg)
                if qg + 2 < NQB:
                    emit_stage(qg + 2)

    nc.finalize()
    return nc


def _owned_keys(par):
    return np.concatenate(
        [np.arange((2 * i + par) * KC, (2 * i + par) * KC + KC) for i in range(NKLOC)]
    )


def _build_masks(par):
    # diagonal pair of each query block: slot s in {0,1} is relative chunk
    # r = 2s + par; element [k, q] allowed iff 128*r + k <= q
    r = np.array([par, 2 + par])[:, None, None]
    k = np.arange(KC)[None, :, None]
    q = np.arange(TQ)[None, None, :]
    allowed = (KC * r + k) <= q
    return np.where(allowed, np.float32(0.0), np.float32(NEG)).astype(np.float32)


def _get_nc():
    if "nc" not in _cache:
        _cache["nc"] = _build_nc()
    return _cache["nc"]


def _pack_pm(a):
    # [DIN, cols] -> partition-major [128, DIN//128, cols]
    return np.ascontiguousarray(a.reshape(DIN // 128, 128, a.shape[1]).transpose(1, 0, 2))


def _prepare_in_maps(x, Wq, Wk, Wv):
    import ml_dtypes

    f8 = ml_dtypes.float8_e4m3
    bf16 = ml_dtypes.bfloat16

    # fp8 weights: [128, 3, NDC, DOUT]; Wq/Wk prescaled by WS, Wv unscaled
    w8_pack = np.stack(
        [_pack_pm(np.clip(w * s, -240, 240).astype(np.float32))
         for w, s in ((Wq, WS), (Wk, WS), (Wv, 1.0))],
        axis=1,
    ).astype(f8)  # [128, 3, NDC, DOUT]

    wv16_pack = _pack_pm(Wv).reshape(128, NDC * DOUT).astype(bf16)

    in_maps = []
    for c in range(NCORES):
        b, par = c // 2, c % 2
        xbt = np.ascontiguousarray(x[b].T)
        owned = _owned_keys(par)
        m = _build_masks(par)  # [2, 128, 512]
        m16 = np.ascontiguousarray(m.transpose(1, 0, 2)).reshape(128, 2 * TQ)
        wm16 = np.concatenate([m16, np.zeros((128, NDC * DOUT), np.float32)], axis=1)
        wm16 = wm16.astype(bf16)
        wm16[:, 2 * TQ:] = wv16_pack
        in_maps.append({
            "xb8": _pack_pm(xbt).astype(f8),
            "xk8": _pack_pm(np.ascontiguousarray(xbt[:, owned])).astype(f8),
            "xe": _pack_pm(np.ascontiguousarray(xbt[:, owned[:2 * KC]])).astype(bf16),
            "wm8": w8_pack,
            "wm16": wm16,
        })
    return in_maps


def _gather(results):
    out = np.empty((B, S, DOUT), dtype=np.float32)
    for b in range(B):
        r0, r1 = results[2 * b], results[2 * b + 1]
        num = r0["numT"].astype(np.float64).T + r1["numT"].astype(np.float64).T
        d = r0["den"].astype(np.float64).reshape(-1) + r1["den"].astype(
            np.float64
        ).reshape(-1)
        out[b] = (num / d[:, None]).astype(np.float32)
    return out


def kernel(**inputs):
    from concourse.bass_utils import run_bass_kernel_spmd

    x = np.asarray(inputs["x"], dtype=np.float32)
    Wq = np.asarray(inputs["Wq"], dtype=np.float32)
    Wk = np.asarray(inputs["Wk"], dtype=np.float32)
    Wv = np.asarray(inputs["Wv"], dtype=np.float32)

    nc = _get_nc()
    in_maps = _prepare_in_maps(x, Wq, Wk, Wv)
    res = run_bass_kernel_spmd(nc, in_maps, list(range(NCORES)))
    return _gather(res.results)
